# revision 20
# baseline (speedup 1.0000x reference)
"""GCN (2-layer GCNConv + global_add_pool + dense head) on 8 TRN2 cores.

Strategy (graph/data parallel, per sharding hint):
 - Nodes block-partitioned: core c owns rows [6250c, 6250(c+1)).
 - Edges partitioned by destination block, sorted by (dst window, src half).
 - Per layer: h = x @ W in fp16, hs = h * dinv folded on the Act engine,
   AllGather hs -> full fp16 table in every core's HBM.
 - Aggregation: batched dma_gather (one 512B descriptor per edge) using an
   overlapping-stride table view (elem_step = 1 row, elem = 2 rows) so each
   slot holds its true source row; int16 index range handled by splitting
   the table into lo/hi half views.  Per 128-slot chunk: one one-hot matmul
   (C built on DVE via is_equal) accumulated into the window's PSUM.
 - Self-loop contribution via identity matmul of hs; bias via K=1 matmul of
   sqrt(deg) x b.  Epilogue relu(po * dinv) on Act; layer-1 epilogue also
   transposes x2 (PE) and immediately runs the layer-2 feature matmul.
 - Layer-2 windows accumulate global_add_pool via one-hot graph matmuls;
   pooled partials are scattered to graph rows, AllReduced, and the dense
   head + log_softmax runs redundantly on every core.
"""
import sys

sys.path.insert(0, "/opt/trn_rl_repo")

import math
import numpy as np

import concourse.bacc as bacc
import concourse.bass as bass
import concourse.mybir as mybir
import concourse.tile as tile

P = 128
N_NODES = 50000
N_EDGES = 640000
DIM = 128
DIM_OUT = 64
NUM_GRAPHS = 512
NCORES = 8
NB = N_NODES // NCORES          # 6250 nodes per core
WPC = math.ceil(NB / P)         # 49 windows per core
NBP = WPC * P                   # 6272 padded
HALFP = 25088                   # permuted-subrow split (= 512 * WPC)
TPR = NCORES * P                # 1024 table partition-rows
WGRP = 2                        # windows per gather batch

fp32 = mybir.dt.float32
fp16 = mybir.dt.float16
i16 = mybir.dt.int16


# ---------------------------------------------------------------- host prep
def preprocess(x, edge_index, x_batch):
    src = np.asarray(edge_index[0], dtype=np.int64)
    dst = np.asarray(edge_index[1], dtype=np.int64)
    xb = np.asarray(x_batch, dtype=np.int64)
    x = np.asarray(x, dtype=np.float32)

    edeg = np.bincount(dst, minlength=N_NODES)
    deg = 1.0 + edeg.astype(np.float32)
    dinv = (1.0 / np.sqrt(deg)).astype(np.float32)
    sqd = np.sqrt(deg).astype(np.float32)

    order = np.argsort(dst, kind="stable")
    src_sorted = src[order]
    starts = np.zeros(N_NODES + 1, np.int64)
    np.cumsum(edeg, out=starts[1:])

    # per (core, window, half) edge lists; chunk grid = max over cores
    lists = [[None] * WPC for _ in range(NCORES)]
    cwlo = np.zeros((NCORES, WPC), np.int64)
    cwhi = np.zeros((NCORES, WPC), np.int64)
    for c in range(NCORES):
        b = c * NB
        for w in range(WPC):
            lo, hi = b + w * P, b + min((w + 1) * P, NB)
            srcs_w = src_sorted[starts[lo]:starts[hi]]
            nloc_w = np.repeat(np.arange(hi - lo), edeg[lo:hi])
            # permuted table subrow: node n -> (core, part, window) subrow id
            cc, rr = srcs_w // NB, srcs_w % NB
            pr = (cc * P + rr % P) * WPC + rr // P
            m = pr < HALFP
            lists[c][w] = (pr[m], nloc_w[m], pr[~m] - HALFP, nloc_w[~m])
            cwlo[c, w] = max(1, math.ceil(int(m.sum()) / P))
            cwhi[c, w] = max(1, math.ceil(int((~m).sum()) / P))
    CWlo = cwlo.max(axis=0)
    CWhi = cwhi.max(axis=0)
    TClo = int(CWlo.sum())
    TChi = int(CWhi.sum())
    elo = np.zeros((NCORES, WPC), np.int64)
    ehi = np.zeros((NCORES, WPC), np.int64)
    for c in range(NCORES):
        for w in range(WPC):
            elo[c, w] = len(lists[c][w][0])
            ehi[c, w] = len(lists[c][w][2])
    EMlo = elo.max(axis=0)
    EMhi = ehi.max(axis=0)

    def wrap16(flat):
        # index i -> [i % 16, i // 16], replicated across 128 partitions
        n = len(flat)
        arr = np.zeros((P, n // 16), np.int16)
        arr[:16] = flat.reshape(n // 16, 16).T
        for r in range(1, 8):
            arr[16 * r:16 * (r + 1)] = arr[:16]
        return arr

    per_core = []
    for c in range(NCORES):
        b = c * NB
        streams = {}
        for half, CW, TC, ilo in (("lo", CWlo, TClo, 0), ("hi", CWhi, TChi, 2)):
            idxf = np.zeros(TC * P, np.int16)
            nof = np.full(TC * P, -1.0, np.float32)
            col = 0
            for w in range(WPC):
                s, n = lists[c][w][ilo], lists[c][w][ilo + 1]
                o = col * P
                idxf[o:o + len(s)] = s.astype(np.int16)
                nof[o:o + len(n)] = n.astype(np.float16)
                col += int(CW[w])
            streams[f"idx_{half}"] = wrap16(idxf)
            streams[f"no_{half}"] = nof.reshape(TC, P).T.copy()

        nid = b + np.arange(NBP)
        ok = np.arange(NBP) < NB
        nidc = np.minimum(nid, N_NODES - 1)
        dinv_c = np.where(ok, dinv[nidc], 0.0).astype(np.float32)
        sqd_c = np.where(ok, sqd[nidc], 0.0).astype(np.float16)
        gmin = int(xb[b])
        xbs_c = np.where(ok, xb[nidc] - gmin, 200.0).astype(np.float32)
        assert int(xb[b + NB - 1]) - gmin + 1 <= P
        pools = np.stack(
            [gmin + np.arange(P, dtype=np.float32) - P * b4 for b4 in range(4)],
            axis=1,
        ).astype(np.float32)

        xT = np.zeros((DIM, NBP), np.float16)
        xT[:, :NB] = x[b:b + NB].T.astype(np.float16)

        per_core.append(dict(
            xT=xT,
            dinv2d=dinv_c.reshape(WPC, P).T.copy(),
            sqdT=sqd_c.reshape(1, NBP),
            xbshift=xbs_c.reshape(WPC, P).T.copy(),
            pools=pools,
            **streams,
        ))

    shared = dict(CWlo=CWlo, CWhi=CWhi, TClo=TClo, TChi=TChi,
                  EMlo=EMlo, EMhi=EMhi)
    return per_core, shared


def const_inputs(W1, b1, W2, b2, Wh, bh):
    iota = np.tile(np.arange(P, dtype=np.float32)[None, :], (P, 1))
    return dict(
        iota=iota, iota16=iota.astype(np.float16),
        ident16=np.eye(P, dtype=np.float16),
        ident=np.eye(P, dtype=np.float32),
        W1=np.asarray(W1, np.float16), W2=np.asarray(W2, np.float16),
        Wh=np.asarray(Wh, np.float32),
        b1=np.asarray(b1, np.float16).reshape(1, DIM),
        b2=np.asarray(b2, np.float16).reshape(1, DIM),
        bh=np.asarray(bh, np.float32).reshape(1, DIM_OUT),
        ones512=np.ones((1, NUM_GRAPHS), np.float32),
    )


# ---------------------------------------------------------------- kernel
def build_kernel(shared, single_core=False, wgrp=WGRP):
    CWlo, CWhi = shared["CWlo"], shared["CWhi"]
    TClo, TChi = shared["TClo"], shared["TChi"]
    EMlo, EMhi = shared["EMlo"], shared["EMhi"]

    def trim(CW, EM, w_last, ncols):
        # fetch only up to the max-over-cores real count of the final window
        cw_l = int(CW[w_last])
        r = min(128 * cw_l, -(-max(1, int(EM[w_last])) // 16) * 16)
        return 128 * (ncols - cw_l) + r, r - 128 * (cw_l - 1)

    nc = bacc.Bacc("TRN2", target_bir_lowering=False, debug=False,
                   enable_asserts=False,
                   num_devices=1 if single_core else NCORES)

    # inputs
    d_xT = nc.dram_tensor("xT", [DIM, NBP], fp16, kind="ExternalInput")
    d_idx = {h: nc.dram_tensor(f"idx_{h}", [P, tc * 8], i16,
                               kind="ExternalInput")
             for h, tc in (("lo", TClo), ("hi", TChi))}
    d_no = {h: nc.dram_tensor(f"no_{h}", [P, tc], fp32, kind="ExternalInput")
            for h, tc in (("lo", TClo), ("hi", TChi))}
    d_dinv = nc.dram_tensor("dinv2d", [P, WPC], fp32, kind="ExternalInput")
    d_sqd = nc.dram_tensor("sqdT", [1, NBP], fp16, kind="ExternalInput")
    d_xbs = nc.dram_tensor("xbshift", [P, WPC], fp32, kind="ExternalInput")
    d_pools = nc.dram_tensor("pools", [P, 4], fp32, kind="ExternalInput")
    d_iota = nc.dram_tensor("iota", [P, P], fp32, kind="ExternalInput")
    d_iota16 = nc.dram_tensor("iota16", [P, P], fp16, kind="ExternalInput")
    d_id16 = nc.dram_tensor("ident16", [P, P], fp16, kind="ExternalInput")
    d_id = nc.dram_tensor("ident", [P, P], fp32, kind="ExternalInput")
    d_W = [nc.dram_tensor("W1", [DIM, DIM], fp16, kind="ExternalInput"),
           nc.dram_tensor("W2", [DIM, DIM], fp16, kind="ExternalInput")]
    d_b = [nc.dram_tensor("b1", [1, DIM], fp16, kind="ExternalInput"),
           nc.dram_tensor("b2", [1, DIM], fp16, kind="ExternalInput")]
    d_Wh = nc.dram_tensor("Wh", [DIM, DIM_OUT], fp32, kind="ExternalInput")
    d_bh = nc.dram_tensor("bh", [1, DIM_OUT], fp32, kind="ExternalInput")
    d_ones = nc.dram_tensor("ones512", [1, NUM_GRAPHS], fp32,
                            kind="ExternalInput")

    d_out = nc.dram_tensor("out", [NUM_GRAPHS, DIM_OUT], fp32,
                           kind="ExternalOutput")

    # internal DRAM (table in permuted [core*P+p, w*DIM+f] layout)
    tbl = [nc.dram_tensor(f"table{l}", [TPR + 1, NBP], fp16,
                          addr_space="Shared")
           for l in range(2)]
    ag_in = [nc.dram_tensor(f"ag_in{l}", [P, NBP], fp16) for l in range(2)]
    ar_in = nc.dram_tensor("ar_in", [DIM, NUM_GRAPHS], fp32)
    ar_out = nc.dram_tensor("ar_out", [DIM, NUM_GRAPHS], fp32,
                            addr_space="Shared")

    # gather batches: [(w0, nw, col0_lo, cols_lo, col0_hi, cols_hi)]
    sizes = [1]
    while sum(sizes) + wgrp <= WPC - 4:
        sizes.append(wgrp)
    while sum(sizes) < WPC:
        sizes.append(1)
    assert sum(sizes) == WPC
    batches = []
    clo = chi = 0
    w0 = 0
    for nw in sizes:
        cl = int(CWlo[w0:w0 + nw].sum())
        ch = int(CWhi[w0:w0 + nw].sum())
        batches.append((w0, nw, clo, cl, chi, ch))
        clo += cl
        chi += ch
        w0 += nw

    with tile.TileContext(nc) as tc:
        with tc.tile_pool(name="const", bufs=1) as cst, \
             tc.tile_pool(name="big", bufs=1) as bigp, \
             tc.tile_pool(name="glo", bufs=3) as glo_pool, \
             tc.tile_pool(name="ghi", bufs=3) as ghi_pool, \
             tc.tile_pool(name="cpool", bufs=12) as cpool, \
             tc.tile_pool(name="work", bufs=4) as wk, \
             tc.tile_pool(name="ps_feat", bufs=3, space="PSUM") as ps_feat, \
             tc.tile_pool(name="ps_out", bufs=4, space="PSUM") as ps_out, \
             tc.tile_pool(name="ps_aux", bufs=1, space="PSUM") as ps_aux:

            # ---- constants / inputs to SBUF (feature-phase deps first)
            xT_sb = bigp.tile([DIM, NBP], fp16)
            nc.sync.dma_start(xT_sb[:], d_xT[:, :])
            W_sb = []
            for l in range(2):
                t = cst.tile([DIM, DIM], fp16, name=f"W{l}_sb")
                nc.sync.dma_start(t[:], d_W[l][:, :])
                W_sb.append(t)
            dinv_sb = cst.tile([P, WPC], fp32)
            nc.sync.dma_start(dinv_sb[:], d_dinv[:, :])
            idx_sb = {}
            no_sb = {}
            for h, tc_ in (("lo", TClo), ("hi", TChi)):
                t = bigp.tile([P, tc_ * 8], i16, name=f"idx{h}_sb")
                nc.sync.dma_start(t[:], d_idx[h][:, :])
                idx_sb[h] = t
                t = bigp.tile([P, tc_], fp32, name=f"no{h}_sb")
                nc.sync.dma_start(t[:], d_no[h][:, :])
                no_sb[h] = t
            sqd_sb = cst.tile([1, NBP], fp16)
            nc.sync.dma_start(sqd_sb[:], d_sqd[:, :])
            xbs_sb = cst.tile([P, WPC], fp32)
            nc.sync.dma_start(xbs_sb[:], d_xbs[:, :])
            pools_sb = cst.tile([P, 4], fp32)
            nc.sync.dma_start(pools_sb[:], d_pools[:, :])
            iota_sb = cst.tile([P, P], fp32)
            nc.sync.dma_start(iota_sb[:], d_iota[:, :])
            iota16_sb = cst.tile([P, P], fp16)
            nc.sync.dma_start(iota16_sb[:], d_iota16[:, :])
            id16_sb = cst.tile([P, P], fp16)
            nc.sync.dma_start(id16_sb[:], d_id16[:, :])
            id_sb = cst.tile([P, P], fp32)
            nc.sync.dma_start(id_sb[:], d_id[:, :])
            b_sb = []
            for l in range(2):
                t = cst.tile([1, DIM], fp16, name=f"b{l}_sb")
                nc.sync.dma_start(t[:], d_b[l][:, :])
                b_sb.append(t)
            Wh_sb = cst.tile([DIM, DIM_OUT], fp32)
            nc.sync.dma_start(Wh_sb[:], d_Wh[:, :])
            bh_sb = cst.tile([1, DIM_OUT], fp32)
            nc.sync.dma_start(bh_sb[:], d_bh[:, :])
            ones_sb = cst.tile([1, NUM_GRAPHS], fp32)
            nc.sync.dma_start(ones_sb[:], d_ones[:, :])

            hs_sb = [bigp.tile([P, NBP], fp16, name=f"hs{l}") for l in range(2)]

            # dummy Ln+Exp up front: forces the all-in-one act-func table
            # (natural_log_exp_and_others) to load once, off the critical path
            dum = cst.tile([1, 1], fp32)
            nc.vector.memset(dum[:], 1.0)
            nc.scalar.activation(out=dum[:], in_=dum[:],
                                 func=mybir.ActivationFunctionType.Ln)
            nc.scalar.activation(out=dum[:], in_=dum[:],
                                 func=mybir.ActivationFunctionType.Exp)

            # overlapped-stride gather views: elem = 2 rows, step = 1 row
            gview = {}
            for h, base in (("lo", 0), ("hi", NCORES * P // 2)):
                gview[h] = [
                    bass.AP(tbl[l][base:, :].tensor, tbl[l][base:, :].offset,
                            [[DIM, HALFP + 1], [1, 2 * DIM]])
                    for l in range(2)
                ]

            AGB = [6, 13, 20, 27, 34, 41, 45, 48]

            def ag_write(l, w):
                # flush hs windows to ag_in in groups (big descriptors)
                if w in AGB:
                    w0_ = AGB[AGB.index(w) - 1] + 1 if w != 6 else 0
                    gsl = slice(w0_ * P, (w + 1) * P)
                    nc.sync.dma_start(ag_in[l][:, gsl], hs_sb[l][:, gsl])

            def allgather(l):
                if single_core:
                    nc.sync.dma_start(tbl[l][0:P, :], ag_in[l][:, :])
                else:
                    nc.gpsimd.collective_compute(
                        "AllGather", mybir.AluOpType.bypass,
                        ins=[ag_in[l][:, :]],
                        outs=[tbl[l][0:TPR, :]],
                        replica_groups=[list(range(NCORES))])

            def ag_flush(l, w):
                ag_write(l, w)
                if w == WPC - 1:
                    allgather(l)

            # ---- layer-1 features: hs1 = (x @ W1) * dinv
            for w in range(WPC):
                sl = slice(w * P, (w + 1) * P)
                ph = ps_feat.tile([P, DIM], fp32, space="PSUM", tag="ph")
                nc.tensor.matmul(out=ph[:], lhsT=xT_sb[:, sl], rhs=W_sb[0][:],
                                 start=True, stop=True)
                if w % 2 == 0:
                    nc.vector.tensor_scalar(out=hs_sb[0][:, sl], in0=ph[:],
                                            scalar1=dinv_sb[:, w:w + 1],
                                            scalar2=None,
                                            op0=mybir.AluOpType.mult)
                else:
                    nc.scalar.activation(
                        out=hs_sb[0][:, sl], in_=ph[:],
                        func=mybir.ActivationFunctionType.Copy,
                        scale=dinv_sb[:, w:w + 1])
                ag_flush(0, w)

            # ---- aggregation layers
            ccums_lo = np.concatenate([[0], np.cumsum(CWlo)]).astype(int)
            ccums_hi = np.concatenate([[0], np.cumsum(CWhi)]).astype(int)
            pool_ps = ps_aux.tile([P, DIM], fp32, space="PSUM", tag="aux")

            for l in range(2):
                for (w0, nw, c0l, ncl, c0h, nch) in batches:
                    gt = {}
                    for h, c0, ncols, pool_, tcol in (
                            ("lo", c0l, ncl, glo_pool, TClo),
                            ("hi", c0h, nch, ghi_pool, TChi)):
                        g = pool_.tile([P, ncols, 2 * DIM], fp16, tag="g",
                                       name=f"g{h}_{l}_{w0}")
                        CW, EM = (CWlo, EMlo) if h == "lo" else (CWhi, EMhi)
                        nidx, _ = trim(CW, EM, w0 + nw - 1, ncols)
                        nc.gpsimd.dma_gather(
                            out_ap=g[:, :, :], in_ap=gview[h][l],
                            idxs_ap=idx_sb[h][:, 8 * c0:8 * c0 + nidx // 16],
                            num_idxs=nidx, num_idxs_reg=nidx,
                            elem_size=2 * DIM, elem_step=DIM,
                            single_packet=False)
                        gt[h] = g

                    for w in range(w0, w0 + nw):
                        sl = slice(w * P, (w + 1) * P)
                        po = ps_out.tile([P, DIM], fp32, space="PSUM", tag="po")
                        nc.tensor.matmul(out=po[:],
                                         lhsT=sqd_sb[0:1, sl],
                                         rhs=b_sb[l][:], start=True, stop=False)
                        nc.tensor.matmul(out=po[:], lhsT=id16_sb[:],
                                         rhs=hs_sb[l][:, sl],
                                         start=False, stop=False)
                        for h, c0, ccums in (("lo", c0l, ccums_lo),
                                             ("hi", c0h, ccums_hi)):
                            j0 = int(ccums[w]) - c0
                            cw = int(ccums[w + 1] - ccums[w])
                            last = h == "hi"
                            CW, EM = ((CWlo, EMlo) if h == "lo"
                                      else (CWhi, EMhi))
                            batch_final = w == w0 + nw - 1
                            _, klast = trim(CW, EM, w, cw)
                            for j in range(cw):
                                k = (klast if batch_final and j == cw - 1
                                     else P)
                                C = cpool.tile([P, P], fp16, tag="C")
                                nc.vector.tensor_scalar(
                                    out=C[0:k, :], in0=iota16_sb[0:k, :],
                                    scalar1=no_sb[h][0:k, c0 + j0 + j:
                                                     c0 + j0 + j + 1],
                                    scalar2=None,
                                    op0=mybir.AluOpType.is_equal)
                                nc.tensor.matmul(
                                    out=po[:], lhsT=C[0:k, :],
                                    rhs=gt[h][0:k, j0 + j, 0:DIM],
                                    start=False,
                                    stop=last and (j == cw - 1))
                        # epilogue
                        xn = wk.tile([P, DIM], fp16, tag="xn")
                        nc.scalar.activation(
                            out=xn[:], in_=po[:],
                            func=mybir.ActivationFunctionType.Relu,
                            scale=dinv_sb[:, w:w + 1])
                        if l == 0:
                            ptr = ps_feat.tile([P, DIM], fp16, space="PSUM",
                                               tag="ph")
                            nc.tensor.transpose(out=ptr[:], in_=xn[:],
                                                identity=id16_sb[:])
                            x2t = wk.tile([P, DIM], fp16, tag="x2t")
                            nc.scalar.activation(
                                out=x2t[:], in_=ptr[:],
                                func=mybir.ActivationFunctionType.Copy)
                            ph2 = ps_feat.tile([P, DIM], fp32, space="PSUM",
                                               tag="ph")
                            nc.tensor.matmul(out=ph2[:], lhsT=x2t[:],
                                             rhs=W_sb[1][:],
                                             start=True, stop=True)
                            nc.scalar.activation(
                                out=hs_sb[1][:, sl], in_=ph2[:],
                                func=mybir.ActivationFunctionType.Copy,
                                scale=dinv_sb[:, w:w + 1])
                            ag_flush(1, w)
                        else:
                            Cg = wk.tile([P, P], fp16, tag="Cg")
                            nc.vector.tensor_scalar(
                                out=Cg[:], in0=iota16_sb[:],
                                scalar1=xbs_sb[:, w:w + 1],
                                scalar2=None, op0=mybir.AluOpType.is_equal)
                            nc.tensor.matmul(out=pool_ps[:], lhsT=Cg[:],
                                             rhs=xn[:], start=(w == 0),
                                             stop=(w == WPC - 1))

            # ---- pooling scatter + AllReduce
            pool_sb = wk.tile([P, DIM], fp32)
            nc.vector.tensor_copy(pool_sb[:], pool_ps[:])
            sblkT = wk.tile([P, 4, P], fp32)
            for b4 in range(4):
                S = wk.tile([P, P], fp32, tag="S")
                nc.vector.tensor_scalar(
                    out=S[:], in0=iota_sb[:],
                    scalar1=pools_sb[:, b4:b4 + 1],
                    scalar2=None, op0=mybir.AluOpType.is_equal)
                # [feat x graph-block]: transpose-free pooled output
                pblk = ps_feat.tile([P, P], fp32, space="PSUM", tag="ph")
                nc.tensor.matmul(out=pblk[:], lhsT=pool_sb[:], rhs=S[:],
                                 start=True, stop=True)
                nc.vector.tensor_copy(sblkT[:, b4, :], pblk[:])
            nc.sync.dma_start(ar_in[:, :],
                              sblkT[:].rearrange("p a b -> p (a b)"))
            if single_core:
                nc.sync.dma_start(ar_out[:, :], ar_in[:, :])
            else:
                nc.gpsimd.collective_compute(
                    "AllReduce", mybir.AluOpType.add,
                    ins=[ar_in[:, :]], outs=[ar_out[:, :]],
                    replica_groups=[list(range(NCORES))])

            # ---- head
            pooledT = bigp.tile([P, NUM_GRAPHS], fp32)
            nc.sync.dma_start(pooledT[:], ar_out[:, :])
            # logits per graph-block directly: [128 g x 64 c] matmuls
            lg4 = wk.tile([P, 4, DIM_OUT], fp32)
            e4 = wk.tile([P, 4, DIM_OUT], fp32)
            se4 = wk.tile([P, 4], fp32)
            lse4 = wk.tile([P, 4], fp32)
            o4 = wk.tile([P, 4, DIM_OUT], fp32)
            for b4 in range(4):
                lp = ps_feat.tile([P, DIM_OUT], fp32, space="PSUM", tag="ph")
                nc.tensor.matmul(out=lp[:],
                                 lhsT=pooledT[:, b4 * P:(b4 + 1) * P],
                                 rhs=Wh_sb[:], start=True, stop=False)
                nc.tensor.matmul(out=lp[:], lhsT=ones_sb[0:1, 0:P],
                                 rhs=bh_sb[:], start=False, stop=True)
                nc.vector.tensor_copy(lg4[:, b4, :], lp[:])
            # logits are O(5) here, so exp without max-shift is safe
            nc.scalar.activation(out=e4[:].rearrange("p a b -> p (a b)"),
                                 in_=lg4[:].rearrange("p a b -> p (a b)"),
                                 func=mybir.ActivationFunctionType.Exp)
            nc.vector.tensor_reduce(out=se4[:], in_=e4[:],
                                    op=mybir.AluOpType.add,
                                    axis=mybir.AxisListType.X)
            nc.scalar.activation(out=lse4[:], in_=se4[:],
                                 func=mybir.ActivationFunctionType.Ln)
            for b4 in range(4):
                nc.vector.tensor_scalar(out=o4[:, b4, :], in0=lg4[:, b4, :],
                                        scalar1=lse4[:, b4:b4 + 1],
                                        scalar2=None,
                                        op0=mybir.AluOpType.subtract)
            nc.sync.dma_start(d_out[:, :].rearrange("(b p) f -> p b f", p=P),
                              o4[:])

    nc.compile()
    return nc


# ---------------------------------------------------------------- entry
def kernel(x, edge_index, x_batch, W1, b1, W2, b2, Wh, bh):
    """Full-input GCN kernel: shards nodes/edges across 8 NeuronCores."""
    from concourse.bass_utils import run_bass_kernel_spmd

    per_core, shared = preprocess(x, edge_index, x_batch)
    consts = const_inputs(W1, b1, W2, b2, Wh, bh)
    in_maps = [{**pc, **consts} for pc in per_core]
    nc = build_kernel(shared)
    declared = set()
    for alloc in nc.m.functions[0].allocations:
        if isinstance(alloc, mybir.MemoryLocationSet) and \
                alloc.kind == "ExternalInput":
            declared.add(alloc.memorylocations[0].name)
    in_maps = [{k: v for k, v in m.items() if k in declared} for m in in_maps]
    res = run_bass_kernel_spmd(nc, in_maps, core_ids=list(range(NCORES)))
    return np.asarray(res.results[0]["out"], dtype=np.float32)
